# revision 32
# baseline (speedup 1.0000x reference)
"""Trainium2 Bass kernel for nn_MDNLayer (GNN edge MLP + MDN heads).

Strategy: shard the 500k edges across 8 NeuronCores data-parallel. Each core:
  - gathers ligand/protein node features from the replicated h table via
    indirect DMA (4x128 rows per instruction),
  - transposes the gathered tiles to feature-major via the PE,
  - runs the 3-layer MLP (BN folded into weights host-side) with float32r
    matmuls, K-split across 128-partition subtiles,
  - ELU via the identity elu(u) = max(u, min(e^u, 1) - 1) (Exp on ScalarE,
    min/add on DVE/GpSimd, max-with-PSUM on DVE),
  - MDN heads (pi softmax, sigma/mu elu+const) edge-major,
  - streams [E,30] results back to HBM.
Biases ride as per-partition activation bias / tensor_scalar operands; the
bias chain is pre-folded host-side (c_{i+1} = b_{i+1} + W_{i+1}^T c_i).
"""

import numpy as np

N_NODES = 200000
E_TOT = 500000
D = 128
HID = 512
G = 10
BN_EPS = 1e-5
NCORES = 8
P = 128
TPS = 4              # 128-edge tiles per supertile
E_SUP = P * TPS      # 512 edges per supertile
N_SUPER = 123        # supertiles per core -> 62976 padded edges per core
E_CORE_PAD = N_SUPER * E_SUP
E_CORE = E_TOT // NCORES

_PATCHED = False
_BUILT = {}
TRACE = False           # set True (e.g. from test.py) to capture an NTFF trace
LAST_RESULTS = None     # BassKernelResults of the most recent kernel() call


def _apply_tile_patch():
    """This walrus build accepts at most one sync wait per instruction, but the
    stock TileContext epilogue piles every outstanding semaphore wait onto a
    single SP Drain. Spread the extras onto dedicated single-wait SP nops."""
    global _PATCHED
    if _PATCHED:
        return
    _PATCHED = True

    import concourse.mybir as mybir
    import concourse.tile as tile_mod
    from concourse._compat import not_none as nn

    def _patched_drain_and_barrier(self, tick_clock, wait_clock):
        from concourse.vector_clock import ScopedClock

        nc = self.nc
        drain_inst = nc.sync.drain()
        wait_clock.add_sem_waits(
            drain_inst.ins, ScopedClock({None: tick_clock.global_clock})
        )
        si = drain_inst.ins.sync_info
        if si is not None and si.on_wait is not None and len(si.on_wait) > 1:
            waits = list(si.on_wait)
            keep, extra = waits[0], waits[1:]
            bb = nn(nc.cur_bb).bb
            insts = bb.instructions
            drain_pos = insts.index(drain_inst.ins)
            new_nops = []
            for w in extra:
                nop = nc.sync.nop(nofuse=True, hint="drain_wait_split")
                if nop.ins.sync_info is None:
                    nop.ins.sync_info = mybir.SyncInfo(on_wait=[], on_update=[])
                nop.ins.sync_info.on_wait.append(w)
                insts.remove(nop.ins)
                new_nops.append(nop.ins)
            for i, ni in enumerate(new_nops):
                insts.insert(drain_pos + i, ni)
            si.on_wait.clear()
            si.on_wait.append(keep)

        nc.all_engine_barrier()
        assert self.sems is not None
        popped = nc._tile_sem_poison_stack.pop()
        assert popped is self._sem_poison
        nc.clear_and_free_semaphores(list(self.sems.allocated().values()))
        nc.all_engine_barrier()

    tile_mod.TileContext._drain_and_barrier = _patched_drain_and_barrier


def _build(n_super, n_nodes):
    """Build the per-core bass program. Returns the Bass module."""
    _apply_tile_patch()

    import concourse.bacc as bacc
    import concourse.bass as bass
    import concourse.mybir as mybir
    from concourse.bass import IndirectOffsetOnAxis
    from concourse.masks import make_identity
    from concourse.tile import TileContext

    f32 = mybir.dt.float32
    f32r = mybir.dt.float32r
    i32 = mybir.dt.int32
    AF = mybir.ActivationFunctionType
    OP = mybir.AluOpType

    nc = bacc.Bacc("TRN2", target_bir_lowering=False)

    h = nc.dram_tensor("h", [n_nodes, D], f32, kind="ExternalInput")
    idxl = nc.dram_tensor("idxl", [P, n_super * TPS], i32, kind="ExternalInput")
    idxp = nc.dram_tensor("idxp", [P, n_super * TPS], i32, kind="ExternalInput")
    w0 = nc.dram_tensor("w0", [2, P, HID], f32r, kind="ExternalInput")
    wh = nc.dram_tensor("wh", [2, 4, P, HID], f32r, kind="ExternalInput")
    whead = nc.dram_tensor("whead", [4, P, 30], f32r, kind="ExternalInput")
    cb = nc.dram_tensor("cb", [P, 12], f32, kind="ExternalInput")
    ncb = nc.dram_tensor("ncb", [P, 12], f32, kind="ExternalInput")
    chb = nc.dram_tensor("chb", [P, 120], f32, kind="ExternalInput")
    o = nc.dram_tensor("o", [n_super * E_SUP, 30], f32, kind="ExternalOutput")

    o_v = o.rearrange("(t j p) c -> t p j c", t=n_super, j=TPS, p=P)

    with TileContext(nc) as tc:
        with (
            tc.tile_pool(name="const", bufs=1) as cpool,
            tc.tile_pool(name="gather", bufs=4) as gpool,
            tc.tile_pool(name="xt", bufs=3) as xtpool,
            tc.tile_pool(name="scratch", bufs=6) as spool,
            tc.tile_pool(name="acts", bufs=3) as apool,
            tc.tile_pool(name="heads", bufs=2) as hpool,
            tc.tile_pool(name="ptr", bufs=1, space="PSUM") as ptr_pool,
            tc.tile_pool(name="pl", bufs=6, space="PSUM") as pl_pool,
            tc.tile_pool(name="ph", bufs=1, space="PSUM") as ph_pool,
        ):
            ident = cpool.tile([P, P], f32)
            make_identity(nc, ident[:])

            w0_sb = [cpool.tile_from(w0[k], name=f"w0_{k}") for k in range(2)]
            wh_sb = [[cpool.tile_from(wh[i, k], name=f"wh_{i}_{k}") for k in range(4)] for i in range(2)]
            whead_sb = [cpool.tile_from(whead[k], name=f"whead_{k}") for k in range(4)]
            cb_sb = cpool.tile_from(cb[:, :], name="cb_sb")
            ncb_sb = cpool.tile_from(ncb[:, :], name="ncb_sb")
            chb_sb = cpool.tile_from(chb[:, :], name="chb_sb")
            idxl_sb = cpool.tile_from(idxl[:, :], name="idxl_sb")
            idxp_sb = cpool.tile_from(idxp[:, :], name="idxp_sb")

            def emit_gather_transpose(t):
                xts = []
                for side, idx_sb in ((0, idxl_sb), (1, idxp_sb)):
                    xg = gpool.tile([P, TPS * P], f32, tag=f"xg{side}")
                    for j in range(TPS):
                        col = TPS * t + j
                        nc.gpsimd.indirect_dma_start(
                            out=xg[:, P * j : P * (j + 1)],
                            out_offset=None,
                            in_=h[:, :],
                            in_offset=IndirectOffsetOnAxis(
                                ap=idx_sb[:, col : col + 1], axis=0
                            ),
                        )
                    pt = ptr_pool.tile([P, E_SUP], f32, tag="ptr")
                    for j in range(TPS):
                        nc.tensor.transpose(
                            out=pt[:, P * j : P * (j + 1)],
                            in_=xg[:, P * j : P * (j + 1)],
                            identity=ident[:],
                        )
                    xt = xtpool.tile([P, E_SUP], f32r, tag=f"xt{side}")
                    nc.scalar.activation(out=xt[:], in_=pt[:], func=AF.Copy)
                    xts.append(xt)
                return xts

            def emit_layer(t, li, acts):
                w_tiles = w0_sb if li == 0 else wh_sb[li - 1]
                nxt = []
                for c in range(4):
                    ps = pl_pool.tile([P, E_SUP], f32, tag="pl")
                    K = len(acts)
                    for k in range(K):
                        nc.tensor.matmul(
                            out=ps[:],
                            lhsT=w_tiles[k][:, P * c : P * (c + 1)],
                            rhs=acts[k][:],
                            start=(k == 0),
                            stop=(k == K - 1),
                        )
                    col = li * 4 + c
                    e = spool.tile([P, E_SUP], f32, tag="e")
                    nc.scalar.activation(
                        out=e[:], in_=ps[:], func=AF.Exp,
                        bias=cb_sb[:, col : col + 1], scale=1.0,
                    )
                    m = spool.tile([P, E_SUP], f32, tag="m")
                    ts_eng = nc.gpsimd if (li == 2 and c >= 2) else nc.vector
                    ts_eng.tensor_scalar(
                        out=m[:], in0=e[:],
                        scalar1=1.0, scalar2=ncb_sb[:, col : col + 1],
                        op0=OP.min, op1=OP.add,
                    )
                    s = apool.tile([P, E_SUP], f32r, tag=f"act{li}_{c}")
                    nc.vector.tensor_tensor(
                        out=s[:], in0=ps[:], in1=m[:], op=OP.max
                    )
                    nxt.append(s)
                return nxt

            def emit_heads(t, acts):
                ph = ph_pool.tile([P, TPS * 30], f32, tag="ph")
                for j in range(TPS):
                    for k in range(4):
                        nc.tensor.matmul(
                            out=ph[:, 30 * j : 30 * (j + 1)],
                            lhsT=acts[k][:, P * j : P * (j + 1)],
                            rhs=whead_sb[k][:],
                            start=(k == 0),
                            stop=(k == 3),
                        )
                zt = hpool.tile([P, TPS * 30], f32, tag="zt")
                nc.vector.tensor_tensor(out=zt[:], in0=ph[:], in1=chb_sb[:], op=OP.add)
                zv = zt[:].rearrange("p (j c) -> p j c", j=TPS)

                outt = hpool.tile([P, TPS * 30], f32, tag="outt")
                ov = outt[:].rearrange("p (j c) -> p j c", j=TPS)

                epi = hpool.tile([P, TPS * G], f32, tag="epi")
                epi_v = epi[:].rearrange("p (j c) -> p j c", j=TPS)
                nc.scalar.activation(out=epi_v, in_=zv[:, :, 0:G], func=AF.Exp)
                ssum = hpool.tile([P, TPS], f32, tag="ssum")
                nc.vector.reduce_sum(
                    out=ssum[:].rearrange("p (j c) -> p j c", c=1),
                    in_=epi_v, axis=mybir.AxisListType.X,
                )
                rs = hpool.tile([P, TPS], f32, tag="rs")
                nc.vector.reciprocal(out=rs[:], in_=ssum[:])
                for j in range(TPS):
                    nc.gpsimd.tensor_scalar(
                        out=outt[:, 30 * j : 30 * j + G],
                        in0=epi[:, G * j : G * (j + 1)],
                        scalar1=rs[:, j : j + 1], scalar2=None,
                        op0=OP.mult,
                    )

                esm = hpool.tile([P, TPS * 2 * G], f32, tag="esm")
                esm_v = esm[:].rearrange("p (j c) -> p j c", j=TPS)
                nc.scalar.activation(out=esm_v, in_=zv[:, :, G : 3 * G], func=AF.Exp)
                usm = hpool.tile([P, TPS * 2 * G], f32, tag="usm")
                usm_v = usm[:].rearrange("p (j c) -> p j c", j=TPS)
                msm = hpool.tile([P, TPS * 2 * G], f32, tag="msm")
                msm_v = msm[:].rearrange("p (j c) -> p j c", j=TPS)
                for half, cc in ((0, 1.1), (1, 1.0)):
                    sl = slice(G * half, G * (half + 1))
                    nc.vector.tensor_scalar(
                        out=usm_v[:, :, sl], in0=zv[:, :, G + G * half : G + G * (half + 1)],
                        scalar1=cc, scalar2=None, op0=OP.add,
                    )
                    nc.gpsimd.tensor_scalar(
                        out=msm_v[:, :, sl], in0=esm_v[:, :, sl],
                        scalar1=1.0, scalar2=cc - 1.0, op0=OP.min, op1=OP.add,
                    )
                nc.vector.tensor_tensor(
                    out=ov[:, :, G : 3 * G], in0=usm_v, in1=msm_v, op=OP.max
                )
                nc.sync.dma_start(out=o_v[t], in_=ov)

            # 3-stage software pipeline: heads/L2 of t-2, L0/L1 of t-1,
            # gather/transpose of t. Oldest (most-ready) work is emitted
            # first so in-order sequencers rarely park.
            XT = {}
            S0 = {}
            S1 = {}
            for t in range(n_super + 3):
                if t >= 3:
                    a2 = emit_layer(t - 3, 2, S1.pop(t - 3))
                    emit_heads(t - 3, a2)
                if t >= 2 and t - 2 < n_super:
                    S1[t - 2] = emit_layer(t - 2, 1, S0.pop(t - 2))
                if t >= 1 and t - 1 < n_super:
                    S0[t - 1] = emit_layer(t - 1, 0, XT.pop(t - 1))
                if t < n_super:
                    XT[t] = emit_gather_transpose(t)

    nc.compile()
    return nc


def _prep_host(inputs):
    """Fold BN into weights, build bias chain and per-core index layouts."""
    h = np.ascontiguousarray(inputs["h"], dtype=np.float32)
    pl_edge = np.asarray(inputs["pl_edge"])
    W0 = np.asarray(inputs["W0"], dtype=np.float32)
    b0 = np.asarray(inputs["b0"], dtype=np.float32)
    Wh = np.asarray(inputs["Wh"], dtype=np.float32)
    bh = np.asarray(inputs["bh"], dtype=np.float32)
    gam = np.asarray(inputs["bn_gamma"], dtype=np.float32)
    bet = np.asarray(inputs["bn_beta"], dtype=np.float32)
    mu = np.asarray(inputs["bn_mean"], dtype=np.float32)
    var = np.asarray(inputs["bn_var"], dtype=np.float32)

    s = (gam / np.sqrt(var + BN_EPS)).astype(np.float64)
    W0p = W0.astype(np.float64) * s[0]
    b0p = (b0.astype(np.float64) - mu[0]) * s[0] + bet[0]
    W1p = Wh[0].astype(np.float64) * s[1]
    b1p = (bh[0].astype(np.float64) - mu[1]) * s[1] + bet[1]
    W2p = Wh[1].astype(np.float64) * s[2]
    b2p = (bh[1].astype(np.float64) - mu[2]) * s[2] + bet[2]

    c0 = b0p
    c1 = b1p + W1p.T @ c0
    c2 = b2p + W2p.T @ c1
    Whead = np.concatenate(
        [inputs["W_pi"], inputs["W_sigma"], inputs["W_mu"]], axis=1
    ).astype(np.float64)
    bhead = np.concatenate(
        [inputs["b_pi"], inputs["b_sigma"], inputs["b_mu"]]
    ).astype(np.float64)
    ch = bhead + Whead.T @ c2

    w0_dev = np.ascontiguousarray(
        W0p.reshape(2, P, HID).astype(np.float32)
    )
    wh_dev = np.ascontiguousarray(
        np.stack([W1p.reshape(4, P, HID), W2p.reshape(4, P, HID)]).astype(np.float32)
    )
    whead_dev = np.ascontiguousarray(Whead.reshape(4, P, 30).astype(np.float32))

    cmat = np.stack([c0, c1, c2]).astype(np.float32)       # [3, 512]
    cb = np.ascontiguousarray(
        cmat.reshape(3, 4, P).reshape(12, P).T.astype(np.float32)  # [128, 12]
    )
    ncb = np.ascontiguousarray((-1.0 - cb).astype(np.float32))
    chb = np.ascontiguousarray(
        np.tile(ch.astype(np.float32), TPS)[None, :].repeat(P, axis=0)
    )

    # per-core padded, transposed index layouts
    idx_l_all = pl_edge[1].astype(np.int32)
    idx_p_all = pl_edge[0].astype(np.int32)
    idxl_cores, idxp_cores = [], []
    for c in range(NCORES):
        sl = slice(c * E_CORE, (c + 1) * E_CORE)
        for idx_all, dst in ((idx_l_all, idxl_cores), (idx_p_all, idxp_cores)):
            v = np.zeros(E_CORE_PAD, dtype=np.int32)
            v[:E_CORE] = idx_all[sl]
            dst.append(np.ascontiguousarray(v.reshape(-1, P).T))  # [128, n_super*4]

    shared = dict(h=h, w0=w0_dev, wh=wh_dev, whead=whead_dev, cb=cb, ncb=ncb, chb=chb)
    return shared, idxl_cores, idxp_cores


def kernel(**inputs):
    from concourse.bass_utils import run_bass_kernel_spmd

    key = (N_SUPER, N_NODES)
    if key not in _BUILT:
        _BUILT[key] = _build(N_SUPER, N_NODES)
    nc = _BUILT[key]

    shared, idxl_cores, idxp_cores = _prep_host(inputs)
    in_maps = [
        dict(shared, idxl=idxl_cores[c], idxp=idxp_cores[c]) for c in range(NCORES)
    ]
    global LAST_RESULTS
    res = run_bass_kernel_spmd(
        nc, in_maps, core_ids=list(range(NCORES)), trace=TRACE
    )
    LAST_RESULTS = res

    out = np.concatenate(
        [res.results[c]["o"][:E_CORE] for c in range(NCORES)], axis=0
    )
    pi = np.ascontiguousarray(out[:, 0:G])
    sigma = np.ascontiguousarray(out[:, G : 2 * G])
    mu = np.ascontiguousarray(out[:, 2 * G : 3 * G])
    dist = np.asarray(inputs["pl_dist"], dtype=np.float32).reshape(-1, 1)
    return (pi, sigma, mu, dist)
